# revision 8
# baseline (speedup 1.0000x reference)
"""Trainium2 Bass kernel for nn_ConvFilter (geometric-series conv filter).

Math (per batch b, output position l, feature f):
    t[o,l]  = sum_{i,k} conv_w[o,i,k] * x[l+k,i]          (valid conv, L=S-K+1)
    tau     = sigmoid(t + bias)
    out     = (sum_i tau^(7-i) * x[l+i,f]) / (sum_i tau^i)

Implementation (v2 — fp16 end-to-end on device):
  * host pre-transposes x to [feature, seq] fp16 (aligned + 1-shifted copy
    so every even/odd window stays 4-byte aligned for the DVE 2x mode);
    weights converted to fp16 on host. The SAME fp16 x tiles feed both the
    PE conv (fp16 matmul, full rate) and the DVE elementwise chain.
  * conv: 16 accumulating fp16 matmuls per 512-wide l-tile; two overlapping
    l-tiles (0 and L-512) per output-feature block; PSUM fp32.
  * tau = sigmoid(psum+bias) on ACT (fp16 out); T2 = tau^2, T4 = T2^2 on ACT.
  * numerator via Estrin in fp16 2x mode on DVE:
        q_j = tau*x_{2j} + x_{2j+1}
        N   = (q0*T2 + q1)*T4 + (q2*T2 + q3)
  * 1/denominator as ONE custom DVE op: r = p5(tau) where p5 is the degree-5
    relative-minimax fit of 1/((1+t)(1+t^2)(1+t^4)) on [0,1] with p(0)=1,
    p'(0)=-1 pinned (max rel err 1.4e-3). out = N * r in fp16.
  * output DMA'd as fp16; host converts to fp32.
  * data-parallel over batch: 8 batches/core on 8 cores, weights replicated.
"""

import numpy as np
from contextlib import ExitStack

import concourse.bass as bass
import concourse.tile as tile
from concourse import bacc, mybir
from concourse.bass_utils import run_bass_kernel_spmd
from concourse import dve_ops
from concourse.dve_ops import DveOp
from concourse.dve_spec import (
    Spec, Src0, Src1, C0, C1, C2, One, lower, _has_src1,
)
from concourse.dve_uop import DveOpSpec

B, S, F, K = 64, 1024, 256, 8
L = S - K + 1  # 1017
NCORES = 8
BPC = B // NCORES
P = 128
NFB = F // P  # 2 feature blocks
LT = 512      # matmul l-tile width (one PSUM bank)
LE = L + 1    # even fp16 elementwise width (DVE 2x mode needs even counts)
W2 = NFB * S  # 2048: both feature blocks side by side

# degree-3 relative-minimax fit of 1/((1+t)(1+t^2)(1+t^4)) on [0,1] with
# p(0)=1 pinned; max rel err 4.78e-3. Degree 3 (not 4) so the fused
# out = nh * p(tau) op fits the 8-stage DVE pipeline (7 stages).
FP_C1 = -0.93222519
FP_C2 = -0.32912755
FP_C3 = 0.38597704


def _register_op(name, spec, subdim=False):
    for existing in dve_ops.OPS:
        if existing.name == name:
            return existing
    shas = {}
    for ver in ("v3", "v4"):
        tmp = DveOpSpec(name=name, opcode=0, uops=lower(spec, ver=ver),
                        rd1_en=_has_src1(spec))
        shas[ver] = tmp.sha(ver)
    op = DveOp(name, spec, subdim=subdim, uops_sha=shas)
    dve_ops.OPS.append(op)
    dve_ops.CUSTOM_DVE_SPECS[name] = spec
    dve_ops._SUB_OPCODE_FOR_NAME[name] = (
        dve_ops._CUSTOM_DVE_ROW_BASE + len(dve_ops.OPS) - 1
    )
    assert dve_ops._SUB_OPCODE_FOR_NAME[name] < 0x20
    return op


def _get_fuseout_op():
    # out = in1 * (1 + t(c1 + t(c2 + c3 t)))  with t = in0 (tau), in1 = nh.
    # 7 ALU stages: the deg-3 Horner (6) + the final multiply (1).
    t = Src0
    h = C2 * t
    h = C1 + h
    h = h * t
    h = C0 + h
    h = h * t
    p = One + h
    body = p * Src1

    def _ref(in0, in1, s0, s1, imm2):
        tt = in0.astype(np.float32)
        p = 1.0 + tt * (s0 + tt * (s1 + imm2 * tt))
        return (in1.astype(np.float32) * p).astype(np.float32)

    spec = Spec(body=body, reference=_ref)
    return _register_op("ANT_CF_FUSEOUT3", spec)


def build_module():
    FUSEOUT = _get_fuseout_op()
    f32 = mybir.dt.float32
    f16 = mybir.dt.float16
    TT = mybir.AluOpType
    SIG = mybir.ActivationFunctionType.Sigmoid
    SQU = mybir.ActivationFunctionType.Square

    nc = bacc.Bacc("TRN2", target_bir_lowering=False, debug=False,
                   enable_asserts=False, num_devices=NCORES)

    xh_d = nc.dram_tensor("xh", [BPC, P, W2], f16, kind="ExternalInput").ap()
    xo_d = nc.dram_tensor("xo", [BPC, P, W2], f16, kind="ExternalInput").ap()
    # weights packed host-side: wt[p, (ic*K + k)*F + o] = conv_w[o, ic*P+p, k]
    wt_d = nc.dram_tensor("wt", [P, NFB * K * F], f16, kind="ExternalInput").ap()
    cb_d = nc.dram_tensor("cb", [F, 1], f32, kind="ExternalInput").ap()
    yt_d = nc.dram_tensor("yt", [BPC, NFB, P, L], f16, kind="ExternalOutput").ap()

    with tile.TileContext(nc) as tc, ExitStack() as ctx:
        wpool = ctx.enter_context(tc.tile_pool(name="w", bufs=1))
        xpool = ctx.enter_context(tc.tile_pool(name="x", bufs=2))
        tpool = ctx.enter_context(tc.tile_pool(name="t", bufs=2))
        qpool = ctx.enter_context(tc.tile_pool(name="q", bufs=2))
        opool = ctx.enter_context(tc.tile_pool(name="o", bufs=2))
        ppool = ctx.enter_context(tc.tile_pool(name="p", bufs=2, space="PSUM"))

        def load_x(b):
            # x^T fp16, both feature blocks side by side: [128, 2048]
            # xh: aligned copy; xo: 1-left-shifted copy (odd windows).
            # xh split per feature block so the ic=0 matmuls start as soon
            # as the first half lands.
            xh = xpool.tile([P, W2], f16, tag="xh")
            nc.sync.dma_start(xh[:, :S], xh_d[b][:, :S])
            nc.sync.dma_start(xh[:, S:], xh_d[b][:, S:])
            xo = xpool.tile([P, W2], f16, tag="xo")
            nc.sync.dma_start(xo[:], xo_d[b])
            return xh, xo

        # Prologue DMA order: the first sigmoid needs the l0=0 x-chunks of
        # BOTH feature blocks (the conv contracts over all input channels)
        # plus all weights (one packed 1MB DMA, 8KB/partition lines). xo is
        # only read by the DVE chain and streams in behind.
        xh0 = xpool.tile([P, W2], f16, tag="xh")
        xo0 = xpool.tile([P, W2], f16, tag="xo")
        QC = 640  # quarter chunk: covers l0=0 matmuls (cols 0..519) + slack
        nc.sync.dma_start(xh0[:, :QC], xh_d[0][:, :QC])
        nc.sync.dma_start(xh0[:, S:S + QC], xh_d[0][:, S:S + QC])
        w_all = wpool.tile([P, NFB * K * F], f16, tag="w")
        # weights split into 4 chunks in matmul consumption order so the
        # first matmuls stream while the rest land
        WQ = NFB * K * F // 4
        for wi in range(4):
            nc.sync.dma_start(w_all[:, wi * WQ:(wi + 1) * WQ],
                              wt_d[:, wi * WQ:(wi + 1) * WQ])
        nc.sync.dma_start(xo0[:, :QC], xo_d[0][:, :QC])
        nc.sync.dma_start(xh0[:, QC:S], xh_d[0][:, QC:S])
        nc.sync.dma_start(xh0[:, S + QC:], xh_d[0][:, S + QC:])
        nc.sync.dma_start(xo0[:, S:S + QC], xo_d[0][:, S:S + QC])
        nc.sync.dma_start(xo0[:, QC:S], xo_d[0][:, QC:S])
        nc.sync.dma_start(xo0[:, S + QC:], xo_d[0][:, S + QC:])
        x0 = (xh0, xo0)
        bias_sb = wpool.tile([P, NFB], f32, tag="bias")
        nc.sync.dma_start(
            bias_sb[:], cb_d.rearrange("(ob p) one -> p (ob one)", p=P))

        def wslice(k, ic, ob):
            base = (ic * K + k) * F
            return w_all[:, base + ob * P: base + (ob + 1) * P]

        def pair(t, off=0):
            return t[:].rearrange("p (c n) -> p c n", c=2)[:, :, off:off + LE]

        def winview(t):
            # overlapping even windows of a [P, W2] tile:
            # [P, j:4 (stride 2), c:2 (stride S), n:LE (stride 1)]
            ap = t[:].rearrange("p (j c n) -> p j c n", j=4, c=2)
            raw = ap.ap
            raw[1] = [2, 4]
            raw[2] = [S, 2]
            raw[3] = [1, LE]
            ap.ap = raw
            return ap

        def repview(t, nj):
            # [P, W2] tile repeated nj times along a stride-0 j dim
            ap = t[:].rearrange("p (j c n) -> p j c n", j=nj, c=2)
            raw = ap.ap
            raw[1] = [0, nj]
            raw[2] = [S, 2]
            raw[3] = [1, LE]
            ap.ap = raw
            return ap

        def view4(t):   # natural [P, 4, 2, LE] view of a [P, 4*W2] tile
            return t[:].rearrange("p (j c n) -> p j c n", j=4, c=2)[:, :, :, :LE]

        def qsel(t, start):  # j in {start, start+2} of a [P, 4*W2] tile
            return t[:].rearrange(
                "p (a j c n) -> p a j c n", a=2, j=2, c=2
            )[:, :, start, :, :LE]

        def view2(t):   # natural [P, 2, 2, LE] view of a [P, 2*W2] tile
            return t[:].rearrange("p (j c n) -> p j c n", j=2, c=2)[:, :, :, :LE]

        def jslice(t, j):  # single j of a [P, 2*W2] tile -> [P, 2, LE]
            return t[:].rearrange("p (j c n) -> p j c n", j=2, c=2)[:, j, :, :LE]

        prev = None
        for b in range(BPC):
            xh, xo = x0 if b == 0 else load_x(b)

            # conv -> 4 PSUM tiles per batch (2 out-blocks x 2 l-tiles).
            # Batch 0 runs group-major so fb0's tau is ready ~10us earlier;
            # steady state runs weight-major (each LDWEIGHTS feeds 4 MMs).
            pss = {}
            for ob in range(NFB):
                for li, l0 in enumerate((0, L - LT)):
                    pss[(ob, li)] = ppool.tile([P, LT], f32, tag=f"ps{ob}{li}",
                                               name=f"ps{ob}{li}_{b}")

            def mm(ob, li, ic, k):
                l0 = (0, L - LT)[li]
                nc.tensor.matmul(
                    pss[(ob, li)][:],
                    wslice(k, ic, ob),
                    xh[:, ic * S + l0 + k: ic * S + l0 + k + LT],
                    start=(ic == 0 and k == 0),
                    stop=(ic == NFB - 1 and k == K - 1),
                )

            if b == 0:
                for ob in range(NFB):
                    for li in range(2):
                        for ic in range(NFB):
                            for k in range(K):
                                mm(ob, li, ic, k)
            else:
                for ic in range(NFB):
                    for k in range(K):
                        for ob in range(NFB):
                            for li in range(2):
                                mm(ob, li, ic, k)

            # tau (fp16, both obs in one [128, 2048] tile at cols ob*1024)
            tau = tpool.tile([P, W2], f16, tag="tau")
            for ob in range(NFB):
                for li, l0 in enumerate((0, L - LT)):
                    nc.scalar.activation(
                        tau[:, ob * S + l0: ob * S + l0 + LT],
                        pss[(ob, li)][:], SIG,
                        bias=bias_sb[:, ob:ob + 1], scale=1.0)

            t2 = tpool.tile([P, W2], f16, tag="t2")
            t4 = tpool.tile([P, W2], f16, tag="t4")
            u_all = qpool.tile([P, 4 * W2], f16, tag="u4")
            q_all = qpool.tile([P, 4 * W2], f16, tag="q4")
            mh = qpool.tile([P, 2 * W2], f16, tag="mh")

            # Elementwise chain, fp16 2x-mode on DVE, window-batched, split
            # into a HEAD (this iteration) and a TAIL (issued next iteration,
            # after the next batch's u/q — a one-batch software pipeline):
            #   head: u_all[j] = tau * x_{2j}     (DVE, 4 windows in one op)
            #         q_all[j] = u_all[j] + x_{2j+1}   (DVE)
            #         mh = [q0, q2] * T2          (GPSIMD — off the DVE)
            #   tail: hh = mh + [q1, q3]  -> [h0, h2]   (DVE)
            #         m1 = h0 * T4;  nh = m1 + h2       (DVE)
            #         out = nh * p3(tau)   (fused custom op: deg-3 poly of
            #                               1/den + final multiply, DVE 1x)
            # The deferred tail means GPSIMD's mh (slower per element) runs
            # while the DVE does the next batch's u/q, never stalling it.
            def head(fb, n0=0, n1=LE):
                cs = slice(None) if fb is None else slice(fb, fb + 1)
                ns = slice(n0, n1)
                if fb is None:
                    fls = [slice(n0, n1), slice(S + n0, S + n1)]
                else:
                    fls = [slice(fb * S + n0, fb * S + n1)]
                for fl in fls:
                    nc.scalar.activation(t2[:, fl], tau[:, fl], SQU)
                    nc.scalar.activation(t4[:, fl], t2[:, fl], SQU)
                nc.vector.tensor_tensor(view4(u_all)[:, :, cs, ns],
                                        repview(tau, 4)[:, :, cs, ns],
                                        winview(xh)[:, :, cs, ns], TT.mult)
                nc.vector.tensor_tensor(view4(q_all)[:, :, cs, ns],
                                        view4(u_all)[:, :, cs, ns],
                                        winview(xo)[:, :, cs, ns], TT.add)
                nc.gpsimd.tensor_tensor(view2(mh)[:, :, cs, ns],
                                        qsel(q_all, 0)[:, :, cs, ns],
                                        repview(t2, 2)[:, :, cs, ns], TT.mult)

            def tail(bb, tau_, t4_, q_all_, mh_):
                hh = qpool.tile([P, 2 * W2], f16, tag="hh")
                m1 = qpool.tile([P, W2], f16, tag="m1")
                nh = qpool.tile([P, W2], f16, tag="nh")
                oh = opool.tile([P, W2], f16, tag="oh")
                nc.vector.tensor_tensor(view2(hh)[:], view2(mh_)[:],
                                        qsel(q_all_, 1)[:], TT.add)
                nc.vector.tensor_tensor(pair(m1)[:], jslice(hh, 0)[:],
                                        pair(t4_)[:], TT.mult)
                nc.vector.tensor_tensor(pair(nh)[:], pair(m1)[:],
                                        jslice(hh, 1)[:], TT.add)
                nc.vector._custom_dve(
                    FUSEOUT, out=oh[:], in0=tau_[:], in1=nh[:],
                    s0=FP_C1, s1=FP_C2, imm2=FP_C3)
                for ob in range(NFB):
                    nc.sync.dma_start(yt_d[bb, ob], oh[:, ob * S: ob * S + L])

            if b == 0:
                # quarter heads: start the moment each sigmoid group lands.
                # [0,512) needs only the (ob,0) group; [504,1018) the rest.
                for fb in range(NFB):
                    head(fb, 0, LT)
                    head(fb, LT - 8, LE)
            else:
                head(None)
            if prev is not None:
                tail(b - 1, *prev)
            prev = (tau, t4, q_all, mh)
        tail(BPC - 1, *prev)

    nc.compile()
    return nc


_NC = None


def _get_nc():
    global _NC
    if _NC is None:
        _NC = build_module()
    return _NC


def prep_inputs(x, conv_w, conv_b):
    xt = np.ascontiguousarray(
        np.asarray(x).transpose(0, 2, 1)).astype(np.float16)
    xh = xt.reshape(B, NFB * P, S).reshape(B, NFB, P, S)
    # interleave the two feature blocks side by side: [B, P, NFB*S]
    xh = np.ascontiguousarray(xh.transpose(0, 2, 1, 3)).reshape(B, P, W2)
    xo = np.empty_like(xh)
    xo[:, :, :W2 - 1] = xh[:, :, 1:]
    xo[:, :, W2 - 1] = 0
    # pack: wt[p, (ic*K + k)*F + o] = conv_w[o, ic*P+p, k]
    wt = np.asarray(conv_w).astype(np.float16).transpose(1, 2, 0)  # [i, k, o]
    wt = wt.reshape(NFB, P, K, F).transpose(1, 0, 2, 3)            # [p, ic, k, o]
    wt = np.ascontiguousarray(wt).reshape(P, NFB * K * F)
    cb = np.ascontiguousarray(conv_b, dtype=np.float32).reshape(F, 1)
    return xh, xo, wt, cb


def make_in_maps(x, conv_w, conv_b):
    xh, xo, wt, cb = prep_inputs(x, conv_w, conv_b)
    return [
        {"xh": xh[c * BPC:(c + 1) * BPC], "xo": xo[c * BPC:(c + 1) * BPC],
         "wt": wt, "cb": cb}
        for c in range(NCORES)
    ]


def gather_output(results):
    out = np.empty((B, L, F), np.float32)
    for c in range(NCORES):
        yt = results[c]["yt"].astype(np.float32)  # [BPC, NFB, P, L]
        out[c * BPC:(c + 1) * BPC] = (
            yt.transpose(0, 3, 1, 2).reshape(BPC, L, F))
    return out


def kernel(x, conv_w, conv_b):
    nc = _get_nc()
    in_maps = make_in_maps(x, conv_w, conv_b)
    res = run_bass_kernel_spmd(nc, in_maps, core_ids=list(range(NCORES)))
    return gather_output(res.results)

